# revision 16
# baseline (speedup 1.0000x reference)
"""ConvLSTM block Trainium2 kernel (8 NeuronCores).

Sharding: 8 cores = 4 batches x 2 H-halves. Bottom-half cores process their
slab vertically flipped (with kh-flipped conv kernels) so one SPMD program
serves all cores. Per timestep the two halves of a batch exchange one
boundary row of h via a pairwise AllReduce (halo = sum - own, parity-free).

Per-core compute per step: for each of 2 output-channel tiles (128 ch) and
each chunk of 8 output rows (N=512 pixels), one PSUM accumulation group of
9 matmuls: 3x input conv (K=96: 3 row-taps x 32ch; stride-2 column access
via strided APs) + 6x recurrent conv (K=128: 2 row-taps x 64ch using a
row-shifted duplicate copy of h in partitions 64:128). Gates/LSTM/BN run on
ACT+DVE out of PSUM; bias and BN are folded into the activations.
"""
import os
import numpy as np

T, H2, W2, F, CIN = 16, 64, 64, 64, 32
WP, XW, NQ = 66, 130, 8
N_HALVES = 2
R = H2 // N_HALVES
SLAB = 2 * R + 1
NCHUNK = R // NQ
MM_DT = os.environ.get("CONV_LSTM_MM_DT", "bf16")  # bf16 | fp32 | fp32r

_CACHE = {}


def _storage_np_dtype():
    import ml_dtypes
    return ml_dtypes.bfloat16 if MM_DT == "bf16" else np.float32


def _prep_core_inputs(x, W, U, b, gamma, beta, moving_mean, moving_var,
                      bidx, half):
    sdt = _storage_np_dtype()
    flip = (half == 1)

    # x slab [T, CIN, SLAB, XW]; XLA SAME (stride2,k3,even) pads bottom/right
    # only: out row r reads input rows 2r..2r+2 (row/col 128 = zero pad).
    xs = np.zeros((T, CIN, SLAB, XW), np.float32)
    xc = np.ascontiguousarray(x[bidx].transpose(0, 3, 1, 2))  # (T,CIN,128,128)
    if not flip:
        xs[:, :, 0:SLAB, 0:128] = xc[:, :, 0:SLAB, :]
    else:
        # slab[s] = x_global[128 - s]; s=0 is the zero pad row
        xs[:, :, 1:SLAB, 0:128] = xc[:, :, 128 - SLAB + 1:][:, :, ::-1, :]

    Wk = W[::-1].copy() if flip else W
    Uk = U[::-1].copy() if flip else U

    # gate column repack: m0 = (i | o), m1 = (f | g). Keras order in the
    # original 4F axis is i,f,g,o; pairing (i,o) lets one 128-partition
    # Relu activation handle both hard-sigmoid gates.
    perm = np.concatenate([np.arange(0, 64), np.arange(192, 256),
                           np.arange(64, 128), np.arange(128, 192)])
    Wk = Wk[:, :, :, perm]
    Uk = Uk[:, :, :, perm]
    bp = b[perm]

    w3 = np.zeros((96, 768), np.float32)
    ua = np.zeros((128, 768), np.float32)
    ub = np.zeros((128, 768), np.float32)
    for di in range(3):
        for m in range(2):
            g = di * 2 + m
            cols = slice(g * 128, (g + 1) * 128)
            mc = slice(m * 128, (m + 1) * 128)
            for j in range(3):
                w3[32*j:32*j+32, cols] = Wk[j, di, :, mc]
            ua[0:64, cols] = Uk[0, di, :, mc]
            ua[64:128, cols] = Uk[1, di, :, mc]
            ub[0:64, cols] = Uk[2, di, :, mc]

    eps = 1e-3
    scale = (gamma / np.sqrt(moving_var + eps)).astype(np.float32)
    beta2 = (beta - moving_mean * scale).astype(np.float32)
    vecs = np.zeros((128, 8), np.float32)
    vecs[:, 0] = 0.2 * bp[0:128] + 0.5          # (i|o) hard-sigmoid bias
    vecs[0:64, 1] = 0.2 * bp[128:192] + 0.5     # f bias (ps1[0:64])
    vecs[64:128, 2] = bp[192:256]               # g bias (ps1[64:128])
    vecs[0:64, 3] = scale
    vecs[0:64, 4] = beta2
    return {
        "xs": np.ascontiguousarray(xs.astype(sdt)),
        "w3": np.ascontiguousarray(w3.astype(sdt)),
        "ua": np.ascontiguousarray(ua.astype(sdt)),
        "ub": np.ascontiguousarray(ub.astype(sdt)),
        "vecs": vecs,
    }


def _patch_tile_drain():
    """This walrus build encodes at most ONE sync wait per CTRL instruction;
    split the Tile exit drain's waits across SP nops."""
    import bass_rust
    import concourse.tile as tile
    from concourse.vector_clock import ScopedClock
    if getattr(tile.TileContext, "_drain_patched", False):
        return

    def patched(self, tick_clock, wait_clock):
        drain_inst = self.nc.sync.drain()
        wait_clock.add_sem_waits(
            drain_inst.ins, ScopedClock({None: tick_clock.global_clock}))
        si = drain_inst.ins.sync_info
        waits = list(si.on_wait) if si is not None else []
        if len(waits) > 1:
            si.on_wait = waits[:1]
            for w in waits[1:]:
                nop = self.nc.sync.nop()
                nsi = nop.ins.sync_info
                if nsi is None:
                    nop.ins.sync_info = bass_rust.SyncInfo(
                        on_wait=[w], on_update=[])
                else:
                    nsi.on_wait = [w]
        self.nc.all_engine_barrier()
        assert self.sems is not None
        popped = self.nc._tile_sem_poison_stack.pop()
        assert popped is self._sem_poison
        self.nc.clear_and_free_semaphores(list(self.sems.allocated().values()))
        self.nc.all_engine_barrier()

    tile.TileContext._drain_and_barrier = patched
    tile.TileContext._drain_patched = True


def _split_multi_waits(nc, mybir):
    """This walrus build encodes at most one sync wait per instruction;
    move excess waits onto single-wait nops inserted just before."""
    ctr = 0
    for bb in nc.main_func.blocks:
        insts = bb.instructions
        out = []
        changed = False
        for inst in insts:
            si = inst.sync_info
            waits = list(si.on_wait) if si is not None else []
            if len(waits) > 1:
                changed = True
                for w in waits[:-1]:
                    ctr += 1
                    out.append(mybir.InstNoOp(
                        name=f"wsplit-{ctr}",
                        engine=inst.engine,
                        sync_info=mybir.SyncInfo(on_wait=[w], on_update=[]),
                        bass_nofuse=True))
                si.on_wait = [waits[-1]]
            out.append(inst)
        if changed:
            bb.instructions = out


def _build_nc():
    import concourse.bass as bass
    import concourse.mybir as mybir
    import concourse.tile as tile
    _patch_tile_drain()
    dt = mybir.dt
    sdt = dt.bfloat16 if MM_DT == "bf16" else dt.float32
    AF = mybir.ActivationFunctionType

    def mm_ap(ap):
        return ap.bitcast(dt.float32r) if MM_DT == "fp32r" else ap

    nc = bass.Bass()
    xs = nc.dram_tensor("xs", [T, CIN, SLAB, XW], sdt, kind="ExternalInput")
    w3 = nc.dram_tensor("w3", [96, 768], sdt, kind="ExternalInput")
    ua = nc.dram_tensor("ua", [128, 768], sdt, kind="ExternalInput")
    ub = nc.dram_tensor("ub", [128, 768], sdt, kind="ExternalInput")
    vecs = nc.dram_tensor("vecs", [128, 8], dt.float32, kind="ExternalInput")
    y = nc.dram_tensor("y", [T, F, R * W2], dt.float32, kind="ExternalOutput")

    groups = [[0, 1], [2, 3], [4, 5], [6, 7]]

    with tile.TileContext(nc) as tc:
        with (
            tc.tile_pool(name="const", bufs=1) as cpool,
            tc.tile_pool(name="state", bufs=1) as spool,
            tc.tile_pool(name="xp", bufs=2) as xpool,
            tc.tile_pool(name="ps", bufs=6, space="PSUM") as pspool,
            tc.tile_pool(name="mps", bufs=1, space="PSUM") as mpool,
            tc.tile_pool(name="epi", bufs=3) as epool,
            tc.tile_pool(name="halo", bufs=2) as hpool,
            tc.tile_pool(name="dram", bufs=2, space="DRAM") as dpool,
        ):
            w3sb = cpool.tile([96, 768], sdt, tag="w3sb")
            uasb = cpool.tile([128, 768], sdt, tag="uasb")
            ubsb = cpool.tile([128, 768], sdt, tag="ubsb")
            vsb = cpool.tile([128, 8], dt.float32, tag="vsb")
            nc.sync.dma_start(out=w3sb[:], in_=w3[:])
            nc.sync.dma_start(out=uasb[:], in_=ua[:])
            nc.sync.dma_start(out=ubsb[:], in_=ub[:])
            nc.sync.dma_start(out=vsb[:], in_=vecs[:])

            h2 = [spool.tile([128, (R + 2) * WP], sdt, name=f"h2_{i}",
                             tag=f"h2_{i}")
                  for i in range(2)]
            c_sb = spool.tile([64, R * W2], dt.float32, tag="c")
            # h2 memsets split across engines; c_sb is fully written at t=0
            # (f*c term vanishes) so it needs no memset.
            nc.vector.memset(h2[0][:], 0.0)
            nc.gpsimd.memset(h2[1][:], 0.0)

            def load_x3(t):
                x3t = xpool.tile([96, R * XW], sdt, name=f"x3_{t}",
                                 tag="x3")
                x3r = x3t[:].rearrange("p (q w) -> p q w", w=XW)
                nc.sync.dma_start(out=x3r[0:32], in_=xs[t, :, 0:2*R-1:2, :])
                nc.sync.dma_start(out=x3r[32:64], in_=xs[t, :, 1:2*R:2, :])
                nc.sync.dma_start(out=x3r[64:96], in_=xs[t, :, 2:2*R+1:2, :])
                return x3r

            def epilogue(t, hcr, ps0, ps1, row0, nrows, sfx, flat_ps=False):
                """LSTM pointwise + BN for output rows [row0, row0+nrows).
                ps0 holds gates (i|o), ps1 holds (f|g). flat_ps means the
                ps tiles only span these rows (mini path)."""
                a0 = 0 if flat_ps else row0 % NQ
                pcs = slice(a0 * W2, (a0 + nrows) * W2)
                n = nrows * W2
                cs = slice(row0 * W2, (row0 + nrows) * W2)
                # one 128-partition ACT covers both hard-sigmoid gates i,o
                io_t = epool.tile([128, n], dt.float32, tag="io" + sfx)
                nc.scalar.activation(io_t[:], ps0[0:128, pcs], AF.Relu,
                                     bias=vsb[:, 0:1], scale=0.2)
                g_t = epool.tile([64, n], dt.float32, tag="g" + sfx)
                nc.scalar.activation(g_t[:], ps1[64:128, pcs], AF.Tanh,
                                     bias=vsb[64:128, 2:3], scale=1.0)
                # hard-sigmoid clip fused into the gate products:
                # t = (gate min 1.0) * other
                if t == 0:
                    # c_prev == 0: c_new = min(i,1)*tanh(g) directly
                    nc.vector.scalar_tensor_tensor(
                        c_sb[:, cs], io_t[0:64, :], 1.0, g_t[:],
                        mybir.AluOpType.min, mybir.AluOpType.mult)
                else:
                    f_t = epool.tile([64, n], dt.float32, tag="f" + sfx)
                    nc.scalar.activation(f_t[:], ps1[0:64, pcs], AF.Relu,
                                         bias=vsb[0:64, 1:2], scale=0.2)
                    t1 = epool.tile([64, n], dt.float32, tag="t1" + sfx)
                    nc.vector.scalar_tensor_tensor(
                        t1[:], f_t[:], 1.0, c_sb[:, cs],
                        mybir.AluOpType.min, mybir.AluOpType.mult)
                    t2 = epool.tile([64, n], dt.float32, tag="t2" + sfx)
                    nc.vector.scalar_tensor_tensor(
                        t2[:], io_t[0:64, :], 1.0, g_t[:],
                        mybir.AluOpType.min, mybir.AluOpType.mult)
                    nc.vector.tensor_add(c_sb[:, cs], t1[:], t2[:])
                # tanh(c) written into partitions 64:128 so the h product
                # pairs with o (also at base 64)
                tc_t = epool.tile([128, n], dt.float32, tag="tc" + sfx)
                nc.scalar.activation(tc_t[64:128, :], c_sb[:, cs], AF.Tanh)
                hlo = hcr[0:64, row0+1:row0+1+nrows, 1:65]
                nc.vector.scalar_tensor_tensor(
                    hlo,
                    io_t[64:128, :].rearrange("p (a b) -> p a b", b=W2), 1.0,
                    tc_t[64:128, :].rearrange("p (a b) -> p a b", b=W2),
                    mybir.AluOpType.min, mybir.AluOpType.mult)
                if t < T - 1:
                    nc.gpsimd.tensor_copy(
                        out=hcr[64:128, row0:row0+nrows, 1:65], in_=hlo)
                # BN as one gpsimd tensor_scalar with per-partition affine
                yst = epool.tile([64, n], dt.float32, tag="yst" + sfx)
                nc.gpsimd.tensor_scalar(
                    yst[:].rearrange("p (a b) -> p a b", b=W2), hlo,
                    vsb[0:64, 3:4], vsb[0:64, 4:5],
                    mybir.AluOpType.mult, mybir.AluOpType.add)
                nc.gpsimd.dma_start(out=y[t, :, cs], in_=yst[:])

            def mini_block(t, prev_mini, halo_src, mh):
                """Row-31 catch-up for step t (halo-dependent ub taps +
                epilogue), then launch the halo exchange for step t.
                halo_src holds halo(t-1) at position R+1; mh is step t's
                h tile (row 31 written here at position R)."""
                mhr = mh[:].rearrange("p (q w) -> p q w", w=WP)
                if prev_mini is not None:
                    pb = prev_mini  # [128,128]: row-31 partials (m0|m1)
                    hsr = halo_src[:].rearrange("p (q w) -> p q w", w=WP)
                    mps = [mpool.tile([128, 64], dt.float32,
                                      name=f"mps_{t}_{m}", tag=f"mps{m}")
                           for m in range(2)]
                    for m in range(2):
                        for di in range(3):
                            d = di - 1
                            gcol = slice((di*2+m)*128, (di*2+m+1)*128)
                            nc.tensor.matmul(
                                mps[m][:].rearrange("p (a b) -> p a b",
                                                    b=W2),
                                lhsT=mm_ap(ubsb[0:128, gcol]),
                                rhs=mm_ap(hsr[0:128, R+1:R+2, 1+d:65+d]),
                                start=(di == 0), stop=(di == 2))
                    za = epool.tile([128, 64], dt.float32, tag="za")
                    nc.vector.tensor_add(za[:], pb[:, 0:64], mps[0][:])
                    zb = epool.tile([128, 64], dt.float32, tag="zb")
                    nc.vector.tensor_add(zb[:], pb[:, 64:128], mps[1][:])
                    epilogue(t, mhr, za, zb, R - 1, 1, "m", flat_ps=True)
                # halo exchange for step t (skip after last step): row 31
                # just became available at position R of mh
                if t < T - 1:
                    bin_d = dpool.tile([64, 64], sdt, tag="bin")
                    bout_d = dpool.tile([64, 64], sdt, tag="bout")
                    # issue from scalar: sits right behind the mini
                    # epilogue on that queue, so it fires immediately
                    nc.scalar.dma_start(out=bin_d[:],
                                        in_=mhr[0:64, R, 1:65])
                    nc.gpsimd.collective_compute(
                        "AllReduce", mybir.AluOpType.add,
                        replica_groups=groups,
                        ins=[bin_d[:].opt()], outs=[bout_d[:].opt()])
                    bsum = hpool.tile([64, 64], sdt, tag="bsum")
                    nc.gpsimd.dma_start(out=bsum[:], in_=bout_d[:])
                    # halo = sum - own; only [0:64, R+1] is live (the
                    # [64:128, R..R+1] dup region is read solely by zero
                    # rows of ub; memset keeps it finite)
                    nc.gpsimd.tensor_sub(mhr[0:64, R+1, 1:65],
                                         bsum[:], mhr[0:64, R, 1:65])

            x3r = load_x3(0)
            prev_mini = None  # (ps0, ps1) of previous step's boundary chunk
            for t in range(T):
                hc = h2[t % 2]
                hp = h2[(t + 1) % 2]
                hcr = hc[:].rearrange("p (q w) -> p q w", w=WP)
                hpr = hp[:].rearrange("p (q w) -> p q w", w=WP)

                # top-of-step: finish the PREVIOUS step's boundary row 31
                # (its halo has had a full step to cross the AllReduce) and
                # kick off the halo exchange for step t-1. Step t's chunk3
                # ua/ub taps that read h[31](t-1) are ordered after this by
                # the tile dependency tracker.
                if t > 0:
                    # halo(t-2) lives at position R+1 of h2[t % 2] == hc;
                    # step t-1's h tile is h2[(t-1)%2] == hp
                    mini_block(t - 1, prev_mini, hc, hp)
                    prev_mini = None

                # in-order chunks: chunk c's h rows are produced >=2 slots
                # before the next step's consumers need them. Output row 31
                # (the only halo consumer, via the ub tap) is split off into
                # the next step's top-of-step catch-up.
                for ci in range(NCHUNK):
                    q0 = ci * NQ
                    boundary = (ci == NCHUNK - 1) and t > 0
                    pss = [pspool.tile([128, 512], dt.float32,
                                       name=f"ps_{t}_{ci}_{mi}", tag="ps")
                           for mi in range(2)]
                    nmm = 3 if t == 0 else 9
                    for m in range(2):
                        psr = pss[m][:].rearrange("p (a b) -> p a b", b=W2)
                        idx = 0
                        for di in range(3):
                            d = di - 1
                            gcol = slice((di*2+m)*128, (di*2+m+1)*128)
                            nc.tensor.matmul(
                                psr[:],
                                lhsT=mm_ap(w3sb[0:96, gcol]),
                                rhs=mm_ap(x3r[0:96, q0:q0+NQ,
                                              d+1:d+129:2]),
                                start=(idx == 0), stop=(idx == nmm - 1))
                            idx += 1
                        if t > 0:
                            for di in range(3):
                                d = di - 1
                                gcol = slice((di*2+m)*128, (di*2+m+1)*128)
                                nc.tensor.matmul(
                                    psr[:],
                                    lhsT=mm_ap(uasb[0:128, gcol]),
                                    rhs=mm_ap(hpr[0:128, q0:q0+NQ,
                                                  1+d:65+d]),
                                    start=False, stop=False)
                                idx += 1
                            nub = NQ - 1 if boundary else NQ
                            for di in range(3):
                                d = di - 1
                                gcol = slice((di*2+m)*128, (di*2+m+1)*128)
                                nc.tensor.matmul(
                                    psr[:, 0:nub, :],
                                    lhsT=mm_ap(ubsb[0:128, gcol]),
                                    rhs=mm_ap(hpr[0:128, q0+2:q0+2+nub,
                                                  1+d:65+d]),
                                    start=False, stop=(idx == 8))
                                idx += 1

                    ps0, ps1 = pss
                    if boundary:
                        # stash row-31 partial sums in SBUF so the psum
                        # banks recycle without waiting on the halo-gated
                        # row-31 catch-up
                        pb = epool.tile([128, 128], dt.float32, tag="pb")
                        nc.vector.tensor_copy(out=pb[:, 0:64],
                                              in_=ps0[:, 7*W2:8*W2])
                        nc.vector.tensor_copy(out=pb[:, 64:128],
                                              in_=ps1[:, 7*W2:8*W2])
                        epilogue(t, hcr, ps0, ps1, q0, NQ - 1, "b")
                        prev_mini = pb
                    else:
                        epilogue(t, hcr, ps0, ps1, q0, NQ, "")

                    if ci == 0 and t + 1 < T:
                        # x3 prefetch early in the step: its bulk DMA
                        # finishes well before the next step needs it and
                        # clears the queues before the halo readback
                        x3_next = load_x3(t + 1)

                # t=0 has no recurrent term, so chunk3 was computed whole
                # and its boundary row is already in place; its halo
                # exchange launches at the top of step 1.
                if t + 1 < T:
                    x3r = x3_next

            # final catch-up: step T-1's boundary row 31 (halo(T-2) lives
            # in h2[T % 2], step T-1's h tile is h2[(T-1) % 2])
            mini_block(T - 1, prev_mini, h2[T % 2], h2[(T - 1) % 2])
    _split_multi_waits(nc, mybir)
    return nc


def _install_ntff_hook():
    """The image's antenv lacks axon_hooks; synthesize it and register the
    ctypes NTFF profile hook so trace=True works under axon."""
    import sys
    import types
    try:
        from antenv.axon_hooks import get_axon_ntff_profile_hook  # noqa
        return
    except ImportError:
        pass
    mod = types.ModuleType("antenv.axon_hooks")
    mod._hook = None

    def set_axon_ntff_profile_hook(h):
        mod._hook = h

    def get_axon_ntff_profile_hook():
        return mod._hook

    mod.set_axon_ntff_profile_hook = set_axon_ntff_profile_hook
    mod.get_axon_ntff_profile_hook = get_axon_ntff_profile_hook
    sys.modules["antenv.axon_hooks"] = mod
    import antenv
    antenv.axon_hooks = mod
    try:
        from trn_agent_boot.trn_boot import _ntff_profile_via_ctypes
        hook = _ntff_profile_via_ctypes("/opt/axon/libaxon_pjrt.so")
        if hook is not None:
            mod._hook = hook
    except Exception:
        pass


def _get_nc():
    key = (MM_DT,)
    if key not in _CACHE:
        _CACHE[key] = _build_nc()
    return _CACHE[key]


def kernel(x, W, U, b, gamma, beta, moving_mean, moving_var):
    from concourse.bass_utils import run_bass_kernel_spmd
    x = np.asarray(x, np.float32)
    W = np.asarray(W, np.float32)
    U = np.asarray(U, np.float32)
    b = np.asarray(b, np.float32)
    gamma = np.asarray(gamma, np.float32)
    beta = np.asarray(beta, np.float32)
    moving_mean = np.asarray(moving_mean, np.float32)
    moving_var = np.asarray(moving_var, np.float32)
    B = x.shape[0]

    in_maps = []
    for bidx in range(B):
        for half in range(N_HALVES):
            in_maps.append(_prep_core_inputs(
                x, W, U, b, gamma, beta, moving_mean, moving_var, bidx, half))

    nc = _get_nc()
    trace = os.environ.get("BASS_KERNEL_TRACE") == "1"
    if trace:
        _install_ntff_hook()
    res = run_bass_kernel_spmd(nc, in_maps, core_ids=list(range(8)),
                               trace=trace)
    kernel._last_result = res

    out = np.zeros((B, T, H2, W2, F), np.float32)
    ci = 0
    for bidx in range(B):
        for half in range(N_HALVES):
            yc = res.results[ci]["y"].reshape(T, F, R, W2)
            ci += 1
            yc = yc.transpose(0, 2, 3, 1)  # (T, R, W2, F)
            if half == 1:
                yc = yc[:, ::-1, :, :]
                out[bidx, :, 32:64] = yc
            else:
                out[bidx, :, 0:32] = yc
    return out



# revision 22
# speedup vs baseline: 1.1769x; 1.1769x over previous
"""ConvLSTM block Trainium2 kernel (8 NeuronCores).

Sharding: 8 cores = 4 batches x 2 H-halves. Bottom-half cores process their
slab vertically flipped (with kh-flipped conv kernels) so one SPMD program
serves all cores. Per timestep the two halves of a batch exchange one
boundary row of h via a pairwise AllReduce (halo = sum - own, parity-free).

Per-core compute per step: for each of 2 output-channel tiles (128 ch) and
each chunk of 8 output rows (N=512 pixels), one PSUM accumulation group of
9 matmuls: 3x input conv (K=96: 3 row-taps x 32ch; stride-2 column access
via strided APs) + 6x recurrent conv (K=128: 2 row-taps x 64ch using a
row-shifted duplicate copy of h in partitions 64:128). Gates/LSTM/BN run on
ACT+DVE out of PSUM; bias and BN are folded into the activations.
"""
import os
import numpy as np

T, H2, W2, F, CIN = 16, 64, 64, 64, 32
WP, XW, NQ = 66, 130, 8
N_HALVES = 2
R = H2 // N_HALVES
SLAB = 2 * R + 1
NCHUNK = R // NQ
MM_DT = os.environ.get("CONV_LSTM_MM_DT", "bf16")  # bf16 | fp32 | fp32r

_CACHE = {}


def _storage_np_dtype():
    import ml_dtypes
    return ml_dtypes.bfloat16 if MM_DT == "bf16" else np.float32


def _prep_core_inputs(x, W, U, b, gamma, beta, moving_mean, moving_var,
                      bidx, half):
    sdt = _storage_np_dtype()
    flip = (half == 1)

    # x slab [T, CIN, SLAB, XW]; XLA SAME (stride2,k3,even) pads bottom/right
    # only: out row r reads input rows 2r..2r+2 (row/col 128 = zero pad).
    xs = np.zeros((T, CIN, SLAB, XW), np.float32)
    xc = np.ascontiguousarray(x[bidx].transpose(0, 3, 1, 2))  # (T,CIN,128,128)
    if not flip:
        xs[:, :, 0:SLAB, 0:128] = xc[:, :, 0:SLAB, :]
    else:
        # slab[s] = x_global[128 - s]; s=0 is the zero pad row
        xs[:, :, 1:SLAB, 0:128] = xc[:, :, 128 - SLAB + 1:][:, :, ::-1, :]

    Wk = W[::-1].copy() if flip else W
    Uk = U[::-1].copy() if flip else U

    # gate column repack: m0 = (i | o), m1 = (f | g). Keras order in the
    # original 4F axis is i,f,g,o; pairing (i,o) lets one 128-partition
    # Relu activation handle both hard-sigmoid gates.
    perm = np.concatenate([np.arange(0, 64), np.arange(192, 256),
                           np.arange(64, 128), np.arange(128, 192)])
    Wk = Wk[:, :, :, perm]
    Uk = Uk[:, :, :, perm]
    bp = b[perm]

    w3 = np.zeros((96, 768), np.float32)
    ua = np.zeros((128, 768), np.float32)
    ub = np.zeros((128, 768), np.float32)
    for di in range(3):
        for m in range(2):
            g = di * 2 + m
            cols = slice(g * 128, (g + 1) * 128)
            mc = slice(m * 128, (m + 1) * 128)
            for j in range(3):
                w3[32*j:32*j+32, cols] = Wk[j, di, :, mc]
            ua[0:64, cols] = Uk[0, di, :, mc]
            ua[64:128, cols] = Uk[1, di, :, mc]
            ub[0:64, cols] = Uk[2, di, :, mc]

    eps = 1e-3
    scale = (gamma / np.sqrt(moving_var + eps)).astype(np.float32)
    beta2 = (beta - moving_mean * scale).astype(np.float32)
    vecs = np.zeros((128, 8), np.float32)
    vecs[:, 0] = 0.2 * bp[0:128] + 0.5          # (i|o) hard-sigmoid bias
    vecs[0:64, 1] = 0.2 * bp[128:192] + 0.5     # f bias (ps1[0:64])
    vecs[64:128, 2] = bp[192:256]               # g bias (ps1[64:128])
    vecs[0:64, 3] = scale
    vecs[0:64, 4] = beta2
    return {
        "xs": np.ascontiguousarray(xs.astype(sdt)),
        "w3": np.ascontiguousarray(w3.astype(sdt)),
        "ua": np.ascontiguousarray(ua.astype(sdt)),
        "ub": np.ascontiguousarray(ub.astype(sdt)),
        "vecs": vecs,
    }


def _patch_tile_drain():
    """This walrus build encodes at most ONE sync wait per CTRL instruction;
    split the Tile exit drain's waits across SP nops."""
    import bass_rust
    import concourse.tile as tile
    from concourse.vector_clock import ScopedClock
    if getattr(tile.TileContext, "_drain_patched", False):
        return

    def patched(self, tick_clock, wait_clock):
        drain_inst = self.nc.sync.drain()
        wait_clock.add_sem_waits(
            drain_inst.ins, ScopedClock({None: tick_clock.global_clock}))
        si = drain_inst.ins.sync_info
        waits = list(si.on_wait) if si is not None else []
        if len(waits) > 1:
            si.on_wait = waits[:1]
            for w in waits[1:]:
                nop = self.nc.sync.nop()
                nsi = nop.ins.sync_info
                if nsi is None:
                    nop.ins.sync_info = bass_rust.SyncInfo(
                        on_wait=[w], on_update=[])
                else:
                    nsi.on_wait = [w]
        self.nc.all_engine_barrier()
        assert self.sems is not None
        popped = self.nc._tile_sem_poison_stack.pop()
        assert popped is self._sem_poison
        self.nc.clear_and_free_semaphores(list(self.sems.allocated().values()))
        self.nc.all_engine_barrier()

    tile.TileContext._drain_and_barrier = patched
    tile.TileContext._drain_patched = True


def _split_multi_waits(nc, mybir):
    """This walrus build encodes at most one sync wait per instruction;
    move excess waits onto single-wait nops inserted just before."""
    ctr = 0
    for bb in nc.main_func.blocks:
        insts = bb.instructions
        out = []
        changed = False
        for inst in insts:
            si = inst.sync_info
            waits = list(si.on_wait) if si is not None else []
            if len(waits) > 1:
                changed = True
                for w in waits[:-1]:
                    ctr += 1
                    out.append(mybir.InstNoOp(
                        name=f"wsplit-{ctr}",
                        engine=inst.engine,
                        sync_info=mybir.SyncInfo(on_wait=[w], on_update=[]),
                        bass_nofuse=True))
                si.on_wait = [waits[-1]]
            out.append(inst)
        if changed:
            bb.instructions = out


def _build_nc():
    import concourse.bass as bass
    import concourse.mybir as mybir
    import concourse.tile as tile
    _patch_tile_drain()
    dt = mybir.dt
    sdt = dt.bfloat16 if MM_DT == "bf16" else dt.float32
    AF = mybir.ActivationFunctionType

    def mm_ap(ap):
        return ap.bitcast(dt.float32r) if MM_DT == "fp32r" else ap

    nc = bass.Bass()
    xs = nc.dram_tensor("xs", [T, CIN, SLAB, XW], sdt, kind="ExternalInput")
    w3 = nc.dram_tensor("w3", [96, 768], sdt, kind="ExternalInput")
    ua = nc.dram_tensor("ua", [128, 768], sdt, kind="ExternalInput")
    ub = nc.dram_tensor("ub", [128, 768], sdt, kind="ExternalInput")
    vecs = nc.dram_tensor("vecs", [128, 8], dt.float32, kind="ExternalInput")
    y = nc.dram_tensor("y", [T, F, R * W2], dt.float32, kind="ExternalOutput")

    groups = [[0, 1], [2, 3], [4, 5], [6, 7]]

    with tile.TileContext(nc) as tc:
        with (
            tc.tile_pool(name="const", bufs=1) as cpool,
            tc.tile_pool(name="state", bufs=1) as spool,
            tc.tile_pool(name="xp", bufs=2) as xpool,
            tc.tile_pool(name="ps", bufs=6, space="PSUM") as pspool,
            tc.tile_pool(name="mps", bufs=1, space="PSUM") as mpool,
            tc.tile_pool(name="epi", bufs=3) as epool,
            tc.tile_pool(name="halo", bufs=2) as hpool,
            tc.tile_pool(name="dram", bufs=2, space="DRAM") as dpool,
        ):
            w3sb = cpool.tile([96, 768], sdt, tag="w3sb")
            uasb = cpool.tile([128, 768], sdt, tag="uasb")
            ubsb = cpool.tile([128, 768], sdt, tag="ubsb")
            vsb = cpool.tile([128, 8], dt.float32, tag="vsb")
            nc.sync.dma_start(out=w3sb[:], in_=w3[:])
            nc.sync.dma_start(out=uasb[:], in_=ua[:])
            nc.sync.dma_start(out=ubsb[:], in_=ub[:])
            nc.sync.dma_start(out=vsb[:], in_=vecs[:])

            h2 = [spool.tile([128, (R + 2) * WP], sdt, name=f"h2_{i}",
                             tag=f"h2_{i}")
                  for i in range(2)]
            c_sb = spool.tile([64, R * W2], dt.float32, tag="c")
            # h2 memsets split across engines; c_sb is fully written at t=0
            # (f*c term vanishes) so it needs no memset.
            nc.vector.memset(h2[0][:], 0.0)
            nc.gpsimd.memset(h2[1][:], 0.0)

            def load_x3(t):
                x3t = xpool.tile([96, R * XW], sdt, name=f"x3_{t}",
                                 tag="x3")
                x3r = x3t[:].rearrange("p (q w) -> p q w", w=XW)
                nc.sync.dma_start(out=x3r[0:32], in_=xs[t, :, 0:2*R-1:2, :])
                nc.sync.dma_start(out=x3r[32:64], in_=xs[t, :, 1:2*R:2, :])
                nc.sync.dma_start(out=x3r[64:96], in_=xs[t, :, 2:2*R+1:2, :])
                return x3r

            def epilogue(t, hcr, ps0, ps1, row0, nrows, sfx, flat_ps=False):
                """LSTM pointwise + BN for output rows [row0, row0+nrows).
                ps0 holds gates (i|o), ps1 holds (f|g). flat_ps means the
                ps tiles only span these rows (mini path). The mini path
                runs its elementwise ops on gpsimd so the serial halo cycle
                never queues behind the bulk chunk epilogues on vector."""
                # STT/tensor_scalar lower to DVE-only opcodes; Pool only
                # handles plain copies, so all of these stay on vector
                dve = nc.vector
                a0 = 0 if flat_ps else row0 % NQ
                pcs = slice(a0 * W2, (a0 + nrows) * W2)
                n = nrows * W2
                cs = slice(row0 * W2, (row0 + nrows) * W2)
                # one 128-partition ACT covers both hard-sigmoid gates i,o
                io_t = epool.tile([128, n], dt.float32, tag="io" + sfx)
                nc.scalar.activation(io_t[:], ps0[0:128, pcs], AF.Relu,
                                     bias=vsb[:, 0:1], scale=0.2)
                g_t = epool.tile([64, n], dt.float32, tag="g" + sfx)
                nc.scalar.activation(g_t[:], ps1[64:128, pcs], AF.Tanh,
                                     bias=vsb[64:128, 2:3], scale=1.0)
                # hard-sigmoid clip fused into the gate products:
                # t = (gate min 1.0) * other
                if t == 0:
                    # c_prev == 0: c_new = min(i,1)*tanh(g) directly
                    dve.scalar_tensor_tensor(
                        c_sb[:, cs], io_t[0:64, :], 1.0, g_t[:],
                        mybir.AluOpType.min, mybir.AluOpType.mult)
                else:
                    f_t = epool.tile([64, n], dt.float32, tag="f" + sfx)
                    nc.scalar.activation(f_t[:], ps1[0:64, pcs], AF.Relu,
                                         bias=vsb[0:64, 1:2], scale=0.2)
                    t1 = epool.tile([64, n], dt.float32, tag="t1" + sfx)
                    dve.scalar_tensor_tensor(
                        t1[:], f_t[:], 1.0, c_sb[:, cs],
                        mybir.AluOpType.min, mybir.AluOpType.mult)
                    t2 = epool.tile([64, n], dt.float32, tag="t2" + sfx)
                    dve.scalar_tensor_tensor(
                        t2[:], io_t[0:64, :], 1.0, g_t[:],
                        mybir.AluOpType.min, mybir.AluOpType.mult)
                    dve.tensor_add(c_sb[:, cs], t1[:], t2[:])
                # tanh(c) written into partitions 64:128 so the h product
                # pairs with o (also at base 64)
                tc_t = epool.tile([128, n], dt.float32, tag="tc" + sfx)
                nc.scalar.activation(tc_t[64:128, :], c_sb[:, cs], AF.Tanh)
                hlo = hcr[0:64, row0+1:row0+1+nrows, 1:65]
                dve.scalar_tensor_tensor(
                    hlo,
                    io_t[64:128, :].rearrange("p (a b) -> p a b", b=W2), 1.0,
                    tc_t[64:128, :].rearrange("p (a b) -> p a b", b=W2),
                    mybir.AluOpType.min, mybir.AluOpType.mult)
                if t < T - 1:
                    nc.gpsimd.tensor_copy(
                        out=hcr[64:128, row0:row0+nrows, 1:65], in_=hlo)
                # BN as one tensor_scalar with per-partition affine
                # (TensorScalarPtr is DVE-only; Pool rejects it)
                yst = epool.tile([64, n], dt.float32, tag="yst" + sfx)
                nc.vector.tensor_scalar(
                    yst[:].rearrange("p (a b) -> p a b", b=W2), hlo,
                    vsb[0:64, 3:4], vsb[0:64, 4:5],
                    mybir.AluOpType.mult, mybir.AluOpType.add)
                # y store issued from the near-idle sync queue
                nc.sync.dma_start(out=y[t, :, cs], in_=yst[:])

            def mini_block(t, prev_mini, halo_src, mh):
                """Row-31 catch-up for step t (halo-dependent ub taps +
                epilogue), then launch the halo exchange for step t.
                halo_src holds halo(t-1) at position R+1; mh is step t's
                h tile (row 31 written here at position R)."""
                mhr = mh[:].rearrange("p (q w) -> p q w", w=WP)
                if prev_mini is not None:
                    pb = prev_mini  # [128,128]: row-31 partials (m0|m1)
                    hsr = halo_src[:].rearrange("p (q w) -> p q w", w=WP)
                    mps = [mpool.tile([128, 64], dt.float32,
                                      name=f"mps_{t}_{m}", tag=f"mps{m}")
                           for m in range(2)]
                    for m in range(2):
                        for di in range(3):
                            d = di - 1
                            gcol = slice((di*2+m)*128, (di*2+m+1)*128)
                            nc.tensor.matmul(
                                mps[m][:].rearrange("p (a b) -> p a b",
                                                    b=W2),
                                lhsT=mm_ap(ubsb[0:128, gcol]),
                                rhs=mm_ap(hsr[0:128, R+1:R+2, 1+d:65+d]),
                                start=(di == 0), stop=(di == 2))
                    za = epool.tile([128, 64], dt.float32, tag="za")
                    nc.vector.tensor_add(za[:], pb[:, 0:64], mps[0][:])
                    zb = epool.tile([128, 64], dt.float32, tag="zb")
                    nc.vector.tensor_add(zb[:], pb[:, 64:128], mps[1][:])
                    epilogue(t, mhr, za, zb, R - 1, 1, "m", flat_ps=True)
                # halo exchange for step t (skip after last step): row 31
                # just became available at position R of mh
                if t < T - 1:
                    bin_d = dpool.tile([64, 64], sdt, tag="bin")
                    bout_d = dpool.tile([64, 64], sdt, tag="bout")
                    # issue from scalar: sits right behind the mini
                    # epilogue on that queue, so it fires immediately
                    nc.scalar.dma_start(out=bin_d[:],
                                        in_=mhr[0:64, R, 1:65])
                    nc.gpsimd.collective_compute(
                        "AllReduce", mybir.AluOpType.add,
                        replica_groups=groups,
                        ins=[bin_d[:].opt()], outs=[bout_d[:].opt()])
                    bsum = hpool.tile([64, 64], sdt, tag="bsum")
                    nc.gpsimd.dma_start(out=bsum[:], in_=bout_d[:])
                    # halo = sum - own; only [0:64, R+1] is live (the
                    # [64:128, R..R+1] dup region is read solely by zero
                    # rows of ub; memset keeps it finite)
                    nc.gpsimd.tensor_sub(mhr[0:64, R+1, 1:65],
                                         bsum[:], mhr[0:64, R, 1:65])

            x3r = load_x3(0)
            prev_mini = None  # (ps0, ps1) of previous step's boundary chunk
            for t in range(T):
                hc = h2[t % 2]
                hp = h2[(t + 1) % 2]
                hcr = hc[:].rearrange("p (q w) -> p q w", w=WP)
                hpr = hp[:].rearrange("p (q w) -> p q w", w=WP)

                # top-of-step: finish the PREVIOUS step's boundary row 31
                # (its halo has had a full step to cross the AllReduce) and
                # kick off the halo exchange for step t-1. Step t's chunk3
                # ua/ub taps that read h[31](t-1) are ordered after this by
                # the tile dependency tracker.
                if t > 0:
                    # halo(t-2) lives at position R+1 of h2[t % 2] == hc;
                    # step t-1's h tile is h2[(t-1)%2] == hp
                    mini_block(t - 1, prev_mini, hc, hp)
                    prev_mini = None

                # in-order chunks: chunk c's h rows are produced >=2 slots
                # before the next step's consumers need them. Output row 31
                # (the only halo consumer, via the ub tap) is split off into
                # the next step's top-of-step catch-up.
                for ci in range(NCHUNK):
                    q0 = ci * NQ
                    boundary = (ci == NCHUNK - 1) and t > 0
                    pss = [pspool.tile([128, 512], dt.float32,
                                       name=f"ps_{t}_{ci}_{mi}", tag="ps")
                           for mi in range(2)]
                    nmm = 3 if t == 0 else 9
                    for m in range(2):
                        psr = pss[m][:].rearrange("p (a b) -> p a b", b=W2)
                        idx = 0
                        for di in range(3):
                            d = di - 1
                            gcol = slice((di*2+m)*128, (di*2+m+1)*128)
                            nc.tensor.matmul(
                                psr[:],
                                lhsT=mm_ap(w3sb[0:96, gcol]),
                                rhs=mm_ap(x3r[0:96, q0:q0+NQ,
                                              d+1:d+129:2]),
                                start=(idx == 0), stop=(idx == nmm - 1))
                            idx += 1
                        if t > 0:
                            for di in range(3):
                                d = di - 1
                                gcol = slice((di*2+m)*128, (di*2+m+1)*128)
                                nc.tensor.matmul(
                                    psr[:],
                                    lhsT=mm_ap(uasb[0:128, gcol]),
                                    rhs=mm_ap(hpr[0:128, q0:q0+NQ,
                                                  1+d:65+d]),
                                    start=False, stop=False)
                                idx += 1
                            nub = NQ - 1 if boundary else NQ
                            for di in range(3):
                                d = di - 1
                                gcol = slice((di*2+m)*128, (di*2+m+1)*128)
                                nc.tensor.matmul(
                                    psr[:, 0:nub, :],
                                    lhsT=mm_ap(ubsb[0:128, gcol]),
                                    rhs=mm_ap(hpr[0:128, q0+2:q0+2+nub,
                                                  1+d:65+d]),
                                    start=False, stop=(idx == 8))
                                idx += 1

                    ps0, ps1 = pss
                    if boundary:
                        # stash row-31 partial sums in SBUF so the psum
                        # banks recycle without waiting on the halo-gated
                        # row-31 catch-up
                        pb = epool.tile([128, 128], dt.float32, tag="pb")
                        nc.vector.tensor_copy(out=pb[:, 0:64],
                                              in_=ps0[:, 7*W2:8*W2])
                        nc.vector.tensor_copy(out=pb[:, 64:128],
                                              in_=ps1[:, 7*W2:8*W2])
                        epilogue(t, hcr, ps0, ps1, q0, NQ - 1, "b")
                        prev_mini = pb
                    else:
                        epilogue(t, hcr, ps0, ps1, q0, NQ, "")

                    if ci == 0 and t + 1 < T:
                        # x3 prefetch early in the step: its bulk DMA
                        # finishes well before the next step needs it and
                        # clears the queues before the halo readback
                        x3_next = load_x3(t + 1)

                # t=0 has no recurrent term, so chunk3 was computed whole
                # and its boundary row is already in place; its halo
                # exchange launches at the top of step 1.
                if t + 1 < T:
                    x3r = x3_next

            # final catch-up: step T-1's boundary row 31 (halo(T-2) lives
            # in h2[T % 2], step T-1's h tile is h2[(T-1) % 2])
            mini_block(T - 1, prev_mini, h2[T % 2], h2[(T - 1) % 2])
    _split_multi_waits(nc, mybir)
    return nc


def _install_ntff_hook():
    """The image's antenv lacks axon_hooks; synthesize it and register the
    ctypes NTFF profile hook so trace=True works under axon."""
    import sys
    import types
    try:
        from antenv.axon_hooks import get_axon_ntff_profile_hook  # noqa
        return
    except ImportError:
        pass
    mod = types.ModuleType("antenv.axon_hooks")
    mod._hook = None

    def set_axon_ntff_profile_hook(h):
        mod._hook = h

    def get_axon_ntff_profile_hook():
        return mod._hook

    mod.set_axon_ntff_profile_hook = set_axon_ntff_profile_hook
    mod.get_axon_ntff_profile_hook = get_axon_ntff_profile_hook
    sys.modules["antenv.axon_hooks"] = mod
    import antenv
    antenv.axon_hooks = mod
    try:
        from trn_agent_boot.trn_boot import _ntff_profile_via_ctypes
        hook = _ntff_profile_via_ctypes("/opt/axon/libaxon_pjrt.so")
        if hook is not None:
            mod._hook = hook
    except Exception:
        pass


def _get_nc():
    key = (MM_DT,)
    if key not in _CACHE:
        _CACHE[key] = _build_nc()
    return _CACHE[key]


def kernel(x, W, U, b, gamma, beta, moving_mean, moving_var):
    from concourse.bass_utils import run_bass_kernel_spmd
    x = np.asarray(x, np.float32)
    W = np.asarray(W, np.float32)
    U = np.asarray(U, np.float32)
    b = np.asarray(b, np.float32)
    gamma = np.asarray(gamma, np.float32)
    beta = np.asarray(beta, np.float32)
    moving_mean = np.asarray(moving_mean, np.float32)
    moving_var = np.asarray(moving_var, np.float32)
    B = x.shape[0]

    in_maps = []
    for bidx in range(B):
        for half in range(N_HALVES):
            in_maps.append(_prep_core_inputs(
                x, W, U, b, gamma, beta, moving_mean, moving_var, bidx, half))

    nc = _get_nc()
    trace = os.environ.get("BASS_KERNEL_TRACE") == "1"
    if trace:
        _install_ntff_hook()
    res = run_bass_kernel_spmd(nc, in_maps, core_ids=list(range(8)),
                               trace=trace)
    kernel._last_result = res

    out = np.zeros((B, T, H2, W2, F), np.float32)
    ci = 0
    for bidx in range(B):
        for half in range(N_HALVES):
            yc = res.results[ci]["y"].reshape(T, F, R, W2)
            ci += 1
            yc = yc.transpose(0, 2, 3, 1)  # (T, R, W2, F)
            if half == 1:
                yc = yc[:, ::-1, :, :]
                out[bidx, :, 32:64] = yc
            else:
                out[bidx, :, 0:32] = yc
    return out

